# revision 12
# baseline (speedup 1.0000x reference)
"""Trainium2 Bass kernel for nn_Denoiser_73598559584966.

Full-sequence self-attention (Q=K=V, no scaling) over x: [4, 16, 16, 16, 64]
  t = x.reshape(B, 4096, 64); out = softmax(t @ t^T) @ t

Sharding: 8 cores = 4 batches x 2 query-halves. Each core: 2048 queries
vs the full 4096 keys/values of its batch. No collectives.

Device algorithm per core (scores kept transposed: [keys, queries]),
key tiles in packed pairs (ktA rows 0-63 / ktB rows 64-127 of the PE
array so LDWEIGHTS hides under the other half's stream); all matmul
operands bf16 so FWL (fast weight load) can engage:
  S' = k . (A*q)      single-pass bf16, contraction 64; A = 128/ln2 so
                      S' = A*s accumulates fp32 in PSUM.
  P  = exp(s - shift) as bf16, computed two ways, split by key tile:
        ktA half:  activation(Exp, scale=1/A, bias=-shift)   (exact, ScalarE)
        ktB half:    bitcast_bf16(int16(max(S' + B', 0)))      (Schraudolph
                   fast exp on the bf16 grid via VectorE; B' = 128*(127 -
                   shift*log2 e) - C). The ~2% relative error cancels in
                   the softmax ratio (softmax rows here are near-one-hot).
  O^T[128, q] += Vp_kt^T P_kt   bf16 weights [128 keys, 128]: cols 0-63 =
                   V, col 64 = ones (softmax denominator), zero padding to
                   128 columns keeps the weight load FWL-eligible (faster).
shift is per query-chunk (>= rowmax - 30) so exp never overflows; it
cancels exactly in the softmax ratio. Host divides rows 0..63 by row 64
and transposes while gathering shards (O(N*C) marshaling).

Schedule notes: PV of pair p issues after the scores of pair p+1 so the
PE never waits on the exp; the first key-tile group and query chunk are
DMA'd first with PE warmup matmuls covering the wait (a >3.4us PE idle
gap would HAM-rethrottle the PE to 1.2 GHz, and that throttle has been
seen sticking for ~40us).
"""
import numpy as np

B_, D_, H_, W_, C_ = 4, 16, 16, 16, 64
NTOK = D_ * H_ * W_          # 4096 tokens per batch
NQ = NTOK // 2               # 2048 queries per core
NCORES = 8
NKT = NTOK // 128            # 32 key tiles
NPAIR = NKT // 2             # 16 packed key-tile pairs
NCH = 4                      # query chunks per core
CHW = NQ // NCH              # 512 queries per chunk
NG = 4                       # DMA groups over key tiles
GKT = NKT // NG              # 8 key tiles per group

AEXP = 184.66350558899108    # 128 / ln 2  (bf16 Schraudolph scale)
C_SCH = 5.590103149414062    # Schraudolph bias-correction (bf16 grid)
MARGIN = 30.0                # shift = chunk score bound - MARGIN
NWARM = 10                   # PE warmup matmuls under the DMA prefix

_CACHE = {}


def _build_nc():
    import concourse.bacc as bacc
    import concourse.mybir as mybir
    from concourse.tile import TileContext

    f32 = mybir.dt.float32
    i16 = mybir.dt.int16
    bf16 = mybir.dt.bfloat16
    EXP = mybir.ActivationFunctionType.Exp
    ADD = mybir.AluOpType.add
    MAX = mybir.AluOpType.max
    nc = bacc.Bacc("TRN2", target_bir_lowering=False, debug=False)

    qhh = nc.dram_tensor("qhh", [128, NQ], bf16, kind="ExternalInput")
    khi2 = nc.dram_tensor("khi2", [128, NTOK], bf16, kind="ExternalInput")
    vpk = nc.dram_tensor("vpk", [128, NKT * 128], bf16, kind="ExternalInput")
    bsh = nc.dram_tensor("bsh", [128, NCH], f32, kind="ExternalInput")
    bdv = nc.dram_tensor("bdv", [128, NCH], f32, kind="ExternalInput")
    out = nc.dram_tensor("out", [65, NQ], f32, kind="ExternalOutput")

    GW = GKT * 128            # tokens per DMA group
    with TileContext(nc) as tc:
        with (
            tc.tile_pool(name="const", bufs=1) as const,
            tc.tile_pool(name="pp", bufs=4) as pp,
            tc.tile_pool(name="sbo", bufs=2) as sbo,
            tc.tile_pool(name="ps_s", bufs=3, space="PSUM") as ps_s,
            tc.tile_pool(name="ps_o", bufs=2, space="PSUM") as ps_o,
        ):
            # ---- PE + ACT warmup during the DMA prefix ----
            wz = const.tile([128, 512], bf16, tag="wz")
            nc.vector.memset(wz, 0.0)
            wexp = const.tile([128, 1], f32, tag="wexp")
            nc.scalar.activation(wexp, wz[:, 0:1], EXP)  # pull exp table load
            for _ in range(NWARM):
                wps = ps_s.tile([128, 2, CHW], f32, tag="s")
                nc.tensor.matmul(wps[:, 0, :], wz[:, 0:128], wz,
                                 start=True, stop=True)

            # ---- input DMAs: first-chunk operands first, then the rest ----
            qhh_t = const.tile([128, NQ], bf16, tag="qhh")
            khi2_g, vpk_g = [], []
            for g in range(NG):
                kt_ = const.tile([128, GW], bf16, tag=f"khi2_{g}")
                khi2_g.append(kt_)
                kt_ = const.tile([128, GKT * 128], bf16, tag=f"vpk_{g}")
                vpk_g.append(kt_)
            nc.sync.dma_start(out=khi2_g[0][:, 0:256], in_=khi2[:, 0:256])
            nc.sync.dma_start(out=qhh_t[:, 0:CHW], in_=qhh[:, 0:CHW])
            nc.sync.dma_start(out=khi2_g[0][:, 256:GW], in_=khi2[:, 256:GW])
            bsh_t = const.tile([128, NCH], f32, tag="bsh")
            nc.sync.dma_start(out=bsh_t, in_=bsh[:, :])
            bdv_t = const.tile([128, NCH], f32, tag="bdv")
            nc.sync.dma_start(out=bdv_t, in_=bdv[:, :])
            nc.sync.dma_start(out=vpk_g[0], in_=vpk[:, 0:GKT * 128])
            for g in range(1, NG):
                nc.sync.dma_start(
                    out=khi2_g[g], in_=khi2[:, g * GW:(g + 1) * GW])
                nc.sync.dma_start(
                    out=vpk_g[g],
                    in_=vpk[:, g * GKT * 128:(g + 1) * GKT * 128])
                cs = slice(g * CHW, (g + 1) * CHW)
                nc.sync.dma_start(out=qhh_t[:, cs], in_=qhh[:, cs])

            # ---- main loop (packed key-tile pairs; PV trails by one
            # pair so the PE never waits on the exp engines) ----
            for ch in range(NCH):
                qs = slice(ch * CHW, (ch + 1) * CHW)
                o_acc = ps_o.tile([128, CHW], f32, tag="oacc")
                pqueue = []

                def pv(pr, p_t):
                    g = (2 * pr) // GKT
                    for half in range(2):
                        kt = 2 * pr + half
                        lv = (kt - g * GKT) * 128
                        nc.tensor.matmul(
                            o_acc[:, :],
                            vpk_g[g][:, lv:lv + 128],
                            p_t[:, half, :],
                            start=(pr == 0 and half == 0),
                            stop=(pr == NPAIR - 1 and half == 1),
                            skip_group_check=True,
                        )

                for pr in range(NPAIR):
                    ktA, ktB = 2 * pr, 2 * pr + 1
                    g = ktA // GKT
                    lA = (ktA - g * GKT) * 128
                    lB = (ktB - g * GKT) * 128
                    s_t = ps_s.tile([128, 2, CHW], f32, tag="s")
                    # packed pair: ktA on PE rows 0-63, ktB on rows 64-127
                    nc.tensor.matmul(
                        s_t[:, 0, :],
                        khi2_g[g][0:64, lA:lA + 128], qhh_t[0:64, qs],
                        start=True, stop=True, skip_group_check=True,
                    )
                    nc.tensor.matmul(
                        s_t[:, 1, :],
                        khi2_g[g][64:128, lB:lB + 128], qhh_t[64:128, qs],
                        start=True, stop=True, skip_group_check=True,
                    )
                    p_t = pp.tile([128, 2, CHW], bf16, tag="p")
                    # exact exp on ScalarE for the ktA half (PSUM bank 0);
                    # Schraudolph fast exp on VectorE for ktB (bank 1) --
                    # disjoint banks, and each starts after its own scores
                    nc.scalar.activation(
                        p_t[:, 0, :], s_t[:, 0, :], EXP,
                        bias=bsh_t[:, ch:ch + 1], scale=1.0 / AEXP)
                    nc.vector.tensor_scalar(
                        p_t[:, 1, :].bitcast(i16),
                        s_t[:, 1, :],
                        bdv_t[:, ch:ch + 1], 0.0, ADD, MAX)
                    pqueue.append((pr, p_t))
                    if len(pqueue) > 1:
                        pv(*pqueue.pop(0))
                for pp_item in pqueue:
                    pv(*pp_item)
                # ---- ship O^T chunk (normalize + transpose on host);
                # copy on ScalarE, whose slack exceeds VectorE's ----
                o_sb = sbo.tile([65, CHW], f32, tag="osb")
                nc.scalar.copy(o_sb, o_acc[0:65, :])
                nc.sync.dma_start(out=out[:, qs], in_=o_sb)
    nc.compile()
    return nc


def _prep_inputs(x):
    """Host-side shard + operand marshaling. Returns list of 8 in_maps."""
    import ml_dtypes
    bf16 = ml_dtypes.bfloat16
    t = np.ascontiguousarray(x, np.float32).reshape(B_, NTOK, C_)
    in_maps = []
    for b in range(B_):
        kv = t[b]                                   # [4096, 64]
        k_hi = kv.astype(bf16)
        kmax = float(np.linalg.norm(kv.astype(np.float64), axis=1).max())
        khi2 = np.concatenate([k_hi.T, k_hi.T])     # [128, 4096] bf16
        vcols = np.zeros((NTOK, 128), np.float32)
        vcols[:, 0:C_] = kv
        vcols[:, C_] = 1.0
        vpk = np.concatenate(
            [vcols[i * 128:(i + 1) * 128] for i in range(NKT)],
            axis=1).astype(bf16)                    # [128, 32*128]
        for h in range(2):
            q = t[b, h * NQ:(h + 1) * NQ]           # [2048, 64]
            qa = (q.astype(bf16).astype(np.float32)
                  * np.float32(AEXP)).astype(bf16)
            qhh = np.concatenate([qa.T, qa.T])      # [128, 2048] bf16
            shift = np.empty(NCH, np.float64)
            for c in range(NCH):
                qn = np.linalg.norm(
                    q[c * CHW:(c + 1) * CHW].astype(np.float64), axis=1).max()
                shift[c] = qn * kmax - MARGIN
            bsh = np.broadcast_to(
                (-shift).astype(np.float32), (128, NCH)).copy()
            bdv = np.broadcast_to(
                (16256.0 - C_SCH - AEXP * shift).astype(np.float32),
                (128, NCH)).copy()
            in_maps.append({
                "qhh": qhh, "khi2": khi2, "vpk": vpk, "bsh": bsh, "bdv": bdv,
            })
    return in_maps


def run(x, trace=False):
    from concourse.bass_utils import run_bass_kernel_spmd
    if "nc" not in _CACHE:
        _CACHE["nc"] = _build_nc()
    nc = _CACHE["nc"]
    in_maps = _prep_inputs(x)
    res = run_bass_kernel_spmd(
        nc, in_maps, core_ids=list(range(NCORES)), trace=trace,
    )
    full = np.empty((B_, NTOK, C_), np.float32)
    for b in range(B_):
        for h in range(2):
            o = res.results[2 * b + h]["out"]        # [65, 2048]
            full[b, h * NQ:(h + 1) * NQ] = (o[0:C_] / o[C_]).T
    return full.reshape(B_, D_, H_, W_, C_), res


def kernel(x):
    out, _ = run(x, trace=False)
    return out


# revision 13
# speedup vs baseline: 1.1756x; 1.1756x over previous
"""Trainium2 Bass kernel for nn_Denoiser_73598559584966.

Full-sequence self-attention (Q=K=V, no scaling) over x: [4, 16, 16, 16, 64]
  t = x.reshape(B, 4096, 64); out = softmax(t @ t^T) @ t

Sharding: 8 cores = 4 batches x 2 query-halves. Each core: 2048 queries
vs the full 4096 keys/values of its batch. No collectives.

Device algorithm per core (scores kept transposed: [keys, queries]),
key tiles in packed pairs (ktA rows 0-63 / ktB rows 64-127 of the PE
array so LDWEIGHTS hides under the other half's stream); all matmul
operands bf16 so FWL (fast weight load) can engage:
  S' = k . (A*q)      single-pass bf16, contraction 64; A = 128/ln2 so
                      S' = A*s accumulates fp32 in PSUM.
  P  = exp(s - shift) as bf16, computed two ways, split by key tile:
        ktA half:  activation(Exp, scale=1/A, bias=-shift)   (exact, ScalarE)
        ktB half:    bitcast_bf16(int16(max(S' + B', 0)))      (Schraudolph
                   fast exp on the bf16 grid via VectorE; B' = 128*(127 -
                   shift*log2 e) - C). The ~2% relative error cancels in
                   the softmax ratio (softmax rows here are near-one-hot).
  O^T[128, q] += Vp_kt^T P_kt   bf16 weights [128 keys, 128]: cols 0-63 =
                   V, col 64 = ones (softmax denominator), zero padding to
                   128 columns keeps the weight load FWL-eligible (faster).
shift is per query-chunk (>= rowmax - 30) so exp never overflows; it
cancels exactly in the softmax ratio. Host divides rows 0..63 by row 64
and transposes while gathering shards (O(N*C) marshaling).

Schedule notes: PV of pair p issues after the scores of pair p+1 so the
PE never waits on the exp; the first key-tile group and query chunk are
DMA'd first with PE warmup matmuls covering the wait (a >3.4us PE idle
gap would HAM-rethrottle the PE to 1.2 GHz, and that throttle has been
seen sticking for ~40us).
"""
import numpy as np

B_, D_, H_, W_, C_ = 4, 16, 16, 16, 64
NTOK = D_ * H_ * W_          # 4096 tokens per batch
NQ = NTOK // 2               # 2048 queries per core
NCORES = 8
NKT = NTOK // 128            # 32 key tiles
NPAIR = NKT // 2             # 16 packed key-tile pairs
NCH = 4                      # query chunks per core
CHW = NQ // NCH              # 512 queries per chunk
NG = 4                       # DMA groups over key tiles
GKT = NKT // NG              # 8 key tiles per group

AEXP = 184.66350558899108    # 128 / ln 2  (bf16 Schraudolph scale)
C_SCH = 5.590103149414062    # Schraudolph bias-correction (bf16 grid)
MARGIN = 30.0                # shift = chunk score bound - MARGIN
NWARM = 8                    # PE warmup matmuls under the DMA prefix

_CACHE = {}


def _build_nc():
    import concourse.bacc as bacc
    import concourse.mybir as mybir
    from concourse.tile import TileContext

    f32 = mybir.dt.float32
    i16 = mybir.dt.int16
    bf16 = mybir.dt.bfloat16
    EXP = mybir.ActivationFunctionType.Exp
    ADD = mybir.AluOpType.add
    MAX = mybir.AluOpType.max
    nc = bacc.Bacc("TRN2", target_bir_lowering=False, debug=False)

    qhh = nc.dram_tensor("qhh", [128, NQ], bf16, kind="ExternalInput")
    khi2 = nc.dram_tensor("khi2", [128, NTOK], bf16, kind="ExternalInput")
    vpk = nc.dram_tensor("vpk", [128, NKT * 128], bf16, kind="ExternalInput")
    bsh = nc.dram_tensor("bsh", [128, NCH], f32, kind="ExternalInput")
    bdv = nc.dram_tensor("bdv", [128, NCH], f32, kind="ExternalInput")
    out = nc.dram_tensor("out", [65, NQ], f32, kind="ExternalOutput")

    GW = GKT * 128            # tokens per DMA group
    with TileContext(nc) as tc:
        with (
            tc.tile_pool(name="const", bufs=1) as const,
            tc.tile_pool(name="pp", bufs=4) as pp,
            tc.tile_pool(name="sbo", bufs=2) as sbo,
            tc.tile_pool(name="ps_s", bufs=3, space="PSUM") as ps_s,
            tc.tile_pool(name="ps_o", bufs=2, space="PSUM") as ps_o,
        ):
            # ---- PE + ACT warmup during the DMA prefix ----
            wz = const.tile([128, 512], bf16, tag="wz")
            nc.vector.memset(wz, 0.0)
            wexp = const.tile([128, 1], f32, tag="wexp")
            nc.scalar.activation(wexp, wz[:, 0:1], EXP)  # pull exp table load
            for _ in range(NWARM):
                wps = ps_s.tile([128, 2, CHW], f32, tag="s")
                nc.tensor.matmul(wps[:, 0, :], wz[:, 0:128], wz,
                                 start=True, stop=True)

            # ---- input DMAs: first-chunk operands first, then the rest ----
            qhh_t = const.tile([128, NQ], bf16, tag="qhh")
            khi2_g, vpk_g = [], []
            for g in range(NG):
                kt_ = const.tile([128, GW], bf16, tag=f"khi2_{g}")
                khi2_g.append(kt_)
                kt_ = const.tile([128, GKT * 128], bf16, tag=f"vpk_{g}")
                vpk_g.append(kt_)
            nc.sync.dma_start(out=qhh_t[:, 0:CHW], in_=qhh[:, 0:CHW])
            nc.sync.dma_start(out=khi2_g[0][:, 0:256], in_=khi2[:, 0:256])
            nc.sync.dma_start(out=khi2_g[0][:, 256:GW], in_=khi2[:, 256:GW])
            bsh_t = const.tile([128, NCH], f32, tag="bsh")
            nc.sync.dma_start(out=bsh_t, in_=bsh[:, :])
            bdv_t = const.tile([128, NCH], f32, tag="bdv")
            nc.sync.dma_start(out=bdv_t, in_=bdv[:, :])
            nc.sync.dma_start(out=vpk_g[0], in_=vpk[:, 0:GKT * 128])
            for g in range(1, NG):
                nc.sync.dma_start(
                    out=khi2_g[g], in_=khi2[:, g * GW:(g + 1) * GW])
                nc.sync.dma_start(
                    out=vpk_g[g],
                    in_=vpk[:, g * GKT * 128:(g + 1) * GKT * 128])
                cs = slice(g * CHW, (g + 1) * CHW)
                nc.sync.dma_start(out=qhh_t[:, cs], in_=qhh[:, cs])

            # ---- main loop (packed key-tile pairs; PV trails by one
            # pair so the PE never waits on the exp engines) ----
            for ch in range(NCH):
                qs = slice(ch * CHW, (ch + 1) * CHW)
                o_acc = ps_o.tile([128, CHW], f32, tag="oacc")
                pqueue = []

                def pv(pr, p_t):
                    g = (2 * pr) // GKT
                    for half in range(2):
                        kt = 2 * pr + half
                        lv = (kt - g * GKT) * 128
                        nc.tensor.matmul(
                            o_acc[:, :],
                            vpk_g[g][:, lv:lv + 128],
                            p_t[:, half, :],
                            start=(pr == 0 and half == 0),
                            stop=(pr == NPAIR - 1 and half == 1),
                            skip_group_check=True,
                        )

                for pr in range(NPAIR):
                    ktA, ktB = 2 * pr, 2 * pr + 1
                    g = ktA // GKT
                    lA = (ktA - g * GKT) * 128
                    lB = (ktB - g * GKT) * 128
                    s_t = ps_s.tile([128, 2, CHW], f32, tag="s")
                    # packed pair: ktA on PE rows 0-63, ktB on rows 64-127
                    nc.tensor.matmul(
                        s_t[:, 0, :],
                        khi2_g[g][0:64, lA:lA + 128], qhh_t[0:64, qs],
                        start=True, stop=True, skip_group_check=True,
                    )
                    nc.tensor.matmul(
                        s_t[:, 1, :],
                        khi2_g[g][64:128, lB:lB + 128], qhh_t[64:128, qs],
                        start=True, stop=True, skip_group_check=True,
                    )
                    p_t = pp.tile([128, 2, CHW], bf16, tag="p")
                    # exact exp on ScalarE for the ktA half (PSUM bank 0);
                    # Schraudolph fast exp on VectorE for ktB (bank 1) --
                    # disjoint banks, and each starts after its own scores
                    nc.scalar.activation(
                        p_t[:, 0, :], s_t[:, 0, :], EXP,
                        bias=bsh_t[:, ch:ch + 1], scale=1.0 / AEXP)
                    nc.vector.tensor_scalar(
                        p_t[:, 1, :].bitcast(i16),
                        s_t[:, 1, :],
                        bdv_t[:, ch:ch + 1], 0.0, ADD, MAX)
                    pqueue.append((pr, p_t))
                    if len(pqueue) > 1:
                        pv(*pqueue.pop(0))
                for pp_item in pqueue:
                    pv(*pp_item)
                # ---- ship O^T chunk (normalize + transpose on host) ----
                o_sb = sbo.tile([65, CHW], f32, tag="osb")
                nc.vector.tensor_copy(o_sb, o_acc[0:65, :])
                nc.sync.dma_start(out=out[:, qs], in_=o_sb)
    nc.compile()
    return nc


def _prep_inputs(x):
    """Host-side shard + operand marshaling. Returns list of 8 in_maps."""
    import ml_dtypes
    bf16 = ml_dtypes.bfloat16
    t = np.ascontiguousarray(x, np.float32).reshape(B_, NTOK, C_)
    in_maps = []
    for b in range(B_):
        kv = t[b]                                   # [4096, 64]
        k_hi = kv.astype(bf16)
        kmax = float(np.linalg.norm(kv.astype(np.float64), axis=1).max())
        khi2 = np.concatenate([k_hi.T, k_hi.T])     # [128, 4096] bf16
        vcols = np.zeros((NTOK, 128), np.float32)
        vcols[:, 0:C_] = kv
        vcols[:, C_] = 1.0
        vpk = np.concatenate(
            [vcols[i * 128:(i + 1) * 128] for i in range(NKT)],
            axis=1).astype(bf16)                    # [128, 32*128]
        for h in range(2):
            q = t[b, h * NQ:(h + 1) * NQ]           # [2048, 64]
            qa = (q.astype(bf16).astype(np.float32)
                  * np.float32(AEXP)).astype(bf16)
            qhh = np.concatenate([qa.T, qa.T])      # [128, 2048] bf16
            shift = np.empty(NCH, np.float64)
            for c in range(NCH):
                qn = np.linalg.norm(
                    q[c * CHW:(c + 1) * CHW].astype(np.float64), axis=1).max()
                shift[c] = qn * kmax - MARGIN
            bsh = np.broadcast_to(
                (-shift).astype(np.float32), (128, NCH)).copy()
            bdv = np.broadcast_to(
                (16256.0 - C_SCH - AEXP * shift).astype(np.float32),
                (128, NCH)).copy()
            in_maps.append({
                "qhh": qhh, "khi2": khi2, "vpk": vpk, "bsh": bsh, "bdv": bdv,
            })
    return in_maps


def run(x, trace=False):
    from concourse.bass_utils import run_bass_kernel_spmd
    if "nc" not in _CACHE:
        _CACHE["nc"] = _build_nc()
    nc = _CACHE["nc"]
    in_maps = _prep_inputs(x)
    res = run_bass_kernel_spmd(
        nc, in_maps, core_ids=list(range(NCORES)), trace=trace,
    )
    full = np.empty((B_, NTOK, C_), np.float32)
    for b in range(B_):
        for h in range(2):
            o = res.results[2 * b + h]["out"]        # [65, 2048]
            full[b, h * NQ:(h + 1) * NQ] = (o[0:C_] / o[C_]).T
    return full.reshape(B_, D_, H_, W_, C_), res


def kernel(x):
    out, _ = run(x, trace=False)
    return out
